# revision 24
# baseline (speedup 1.0000x reference)
"""Multi-head attention (raw-reshape variant) on 8 trn2 NeuronCores — v2.

Shapes: B=2, S=2048, D=1024, H=16, dh=64.  The reference uses a raw
reshape (B,S,D)->(B,H,S,dh) (NOT a head transpose), so head h only sees
projected rows [128h, 128h+128).  Each (b, h) pair is fully independent:
core c handles b=c//4 and the 4 heads of seq-block c%4.  No collectives.

v2 redesign vs the v1 assembly kernel:
  * Q and K are projected directly TRANSPOSED at full PE efficiency
    (K=128 contraction, M=128 out): psQ[mo] = sum_kc wT[kc][:,128mo:+128].T
    @ xT[kc].  K chunks land verbatim in kt[128,4096] (partition =
    64*(t%2)+d, col = 512*(t//2)+128p+r) and are consumed as S^T lhsT
    slices with NO rearrange.  Q needs a fold rearrange (partition must
    be t'-independent): DVE 4x tensor_copy into qt2, DUPLICATED into both
    partition halves so the S^T rhs can follow the lhsT's t-parity.
  * S^T is emitted in t-PAIRS on opposite 64-row tile positions
    ((0,0)/(64,0) row tiling) so the two matmuls can overlap on the
    128x128 array (contraction is only dh=64).
  * exp on ACT (psum->sbuf fp16, scale=1/8 fused); mask multiply as DVE
    scalar_tensor_tensor (praw*1.0)*mask which qualifies for the 4x_2p
    perf mode (plain tensor_tensor only gets 2x).
  * PV keeps the ones-augmented V trick (psum rows 0:63 accumulate the
    softmax denominator for free, M does not cost cycles).
  * normalize writes stack2 with contraction repacked to (t%2,d) pairs so
    the final projection runs at K=128 (8 accum steps instead of 16).
  * PSUM choreography: tag "stt" (2 x [128,1024] ping/pong) + tag "big"
    (2 x [128,1024]: psO rotation, psF, and interleaved phase-1 tiles)
    = exactly 8 banks.  Projection work for later pairs (K chunks 2/3,
    Q chunks 2/3, V pairs 1..3, final projections) is injected between
    attention pair-groups to hide it under the ACT exp wall.
"""

import numpy as np

import concourse.bass as bass
import concourse.mybir as mybir
import concourse.tile as tile
from concourse import bacc
from concourse.bass_utils import run_bass_kernel_spmd

F32 = mybir.dt.float32
F16 = mybir.dt.float16

B, S, D, H, DH = 2, 2048, 1024, 16, 64
N_CORES = 8
CORE_ROWS = 512
N_PAIRS = 4
EXP_SCALE = 0.125
MULT = mybir.AluOpType.mult
EXP = mybir.ActivationFunctionType.Exp
COPY = mybir.ActivationFunctionType.Copy

_NC = None


def _build_program():
    nc = bacc.Bacc()

    xq_d = nc.dram_tensor("xq", [8, 128, 512], F16, kind="ExternalInput")
    wq_d = nc.dram_tensor("wq", [8, 128, 1024], F16, kind="ExternalInput")
    xk_d = nc.dram_tensor("xk", [8, 128, 512], F16, kind="ExternalInput")
    wk_d = nc.dram_tensor("wk", [8, 128, 1024], F16, kind="ExternalInput")
    xv_d = nc.dram_tensor("xv", [8, 128, 512], F16, kind="ExternalInput")
    wv_d = nc.dram_tensor("wv", [8, 128, 1024], F16, kind="ExternalInput")
    wo_d = nc.dram_tensor("wo2", [128, 8192], F16, kind="ExternalInput")
    maskc_d = nc.dram_tensor("maskc", [S, S], mybir.dt.uint8,
                             kind="ExternalInput")
    out_d = nc.dram_tensor("out", [CORE_ROWS, D], F16, kind="ExternalOutput")

    with tile.TileContext(nc) as tc:
        with tc.tile_pool(name="persist", bufs=1) as persist, \
             tc.tile_pool(name="work", bufs=1) as work, \
             tc.tile_pool(name="ps", bufs=2, space="PSUM") as ps:

            qt2 = persist.tile([128, 8192], F16, tag="qt2", name="qt2")
            kt = persist.tile([128, 4096], F16, tag="kt", name="kt")
            vaug = [persist.tile([128, 2048], F16, tag=f"vaug{p}", name=f"vaug{p}")
                    for p in range(N_PAIRS)]
            mask_sb = [persist.tile([128, 2048], F16, tag=f"mask{t}", name=f"mask{t}")
                       for t in range(16)]
            stk = [persist.tile([128, 1024], F16, tag=f"stk{p}", name=f"stk{p}")
                   for p in range(N_PAIRS)]
            wo_sb = persist.tile([128, 8192], F16, tag="wo", name="wo")

            # preload the exp table while DMAs stream
            pre = persist.tile([1, 8], F32, tag="pre", name="pre")
            pre_o = persist.tile([1, 8], F32, tag="preo", name="preo")
            nc.vector.memset(pre[:, :], 0.0)
            nc.scalar.activation(pre_o[:, :], pre[:, :], EXP)

            # per-partition fp16 ones: keeps every scalar_tensor_tensor
            # operand a 2-byte SBUF AP so the DVE fast path can engage
            ones_sc = persist.tile([128, 1], F16, tag="ones", name="ones")
            nc.vector.memset(ones_sc[:, :], 1.0)

            # junk tiles for HAM warm-up matmuls during the DMA-bound head
            junk = persist.tile([128, 128], F16, tag="junk", name="junk")
            nc.vector.memset(junk[:, :], 0.0)
            warm_ps = ps.tile([128, 1024], F32, tag="big", name="warm")

            def warm_mm(n=1):
                for _ in range(n):
                    nc.tensor.matmul(warm_ps[:, 0:128], lhsT=junk[:, :],
                                     rhs=junk[:, :], start=True, stop=True)

            # ones columns of vaug ([ones(64) | V_t(64)] per 128-col block)
            for p in range(N_PAIRS):
                va3 = vaug[p][:, :].rearrange("a (t c) -> a t c", c=128)
                nc.vector.memset(va3[:, :, 0:64], 1.0)

            # ---------------- DMA queues ----------------
            # Few, large DMAs: the SP sequencer spends ~650ns issuing each
            # dma_start, so 100 small transfers would serialize the input
            # stream on descriptor generation alone.  Weights stage per-mop
            # (the 256 output columns a projection chunk consumes), x/v
            # tensors stage whole with halved transfers for pacing.
            wq_st = {}
            wk_st = {}
            xqst = work.tile([128, 4096], F16, tag="xq", name="xq", bufs=1)
            xkst = work.tile([128, 4096], F16, tag="xk", name="xk", bufs=1)
            xvst = work.tile([128, 4096], F16, tag="xv", name="xv", bufs=1)
            wvst = work.tile([128, 8192], F16, tag="wv", name="wv", bufs=1)

            def dma_w_mop(dst_map, src_d, mop, tagpfx):
                t_ = work.tile([128, 2048], F16, tag=tagpfx, name=f"{tagpfx}s",
                               bufs=2)
                nc.sync.dma_start(
                    out=t_[:, :].rearrange("p (k c) -> p k c", c=256),
                    in_=src_d[:, :, 256 * mop:256 * (mop + 1)].rearrange(
                        "k p c -> p k c"))
                dst_map[mop] = t_

            def dma_x_half(dst, src_d, h, c):
                nc.sync.dma_start(
                    out=dst[:, 4 * c * h:4 * c * (h + 1)].rearrange(
                        "p (k cc) -> p k cc", cc=c),
                    in_=src_d[4 * h:4 * (h + 1)].rearrange("k p c -> p k c"))

            dma_w_mop(wq_st, wq_d, 0, "wq")
            dma_x_half(xqst, xq_d, 0, 512)
            dma_x_half(xqst, xq_d, 1, 512)
            dma_w_mop(wq_st, wq_d, 1, "wq")
            dma_w_mop(wk_st, wk_d, 0, "wk")
            dma_x_half(xkst, xk_d, 0, 512)
            dma_x_half(xkst, xk_d, 1, 512)
            dma_w_mop(wk_st, wk_d, 1, "wk")
            dma_x_half(wvst, wv_d, 0, 1024)
            dma_x_half(wvst, wv_d, 1, 1024)
            dma_x_half(xvst, xv_d, 0, 512)
            dma_x_half(xvst, xv_d, 1, 512)
            dma_w_mop(wk_st, wk_d, 2, "wk")
            dma_w_mop(wk_st, wk_d, 3, "wk")
            dma_w_mop(wq_st, wq_d, 2, "wq")
            dma_w_mop(wq_st, wq_d, 3, "wq")
            nc.sync.dma_start(out=wo_sb[:, :], in_=wo_d[:, :])

            # mask arrives as uint8 (half the DMA bytes) on the SWDGE
            # queue and is expanded to fp16 on whichever engine is idle at
            # that point of the run: ACT/DVE during the projection head,
            # GPSIMD (SBUF-only engine) for the later tiles.
            mu8 = []

            def mask_dma(t):
                m_ = work.tile([128, 2048], mybir.dt.uint8, tag="mu8",
                               name="mu8", bufs=3)
                nc.gpsimd.dma_start(out=m_[:, :], in_=maskc_d[t::16, :])
                mu8.append(m_)

            def mask_expand(t):
                if t < 4:
                    nc.scalar.activation(mask_sb[t][:, :], mu8[t][:, :], COPY)
                elif t < 8:
                    nc.vector.tensor_copy(mask_sb[t][:, :], mu8[t][:, :])
                else:
                    # gpsimd expands must precede the DMA that reuses their
                    # staging slot (same queue) or the queue deadlocks
                    nc.gpsimd.tensor_copy(mask_sb[t][:, :], mu8[t][:, :])

            for t in range(11):
                mask_dma(t)
            for t in range(4, 8):
                mask_expand(t)
            for t in range(11, 16):
                mask_expand(t - 3)
                mask_dma(t)
            for t in range(13, 16):
                mask_expand(t)
            for t in range(4):
                mask_expand(t)

            # ---------------- projection emitters ----------------
            def q_mms(qps, mop, kcs, warm):
                for kc in kcs:
                    for half in range(2):
                        nc.tensor.matmul(
                            qps[:, 512 * half:512 * (half + 1)],
                            lhsT=wq_st[mop][:, 256 * kc + 128 * half:
                                            256 * kc + 128 * (half + 1)],
                            rhs=xqst[:, 512 * kc:512 * (kc + 1)],
                            start=(kc == 0), stop=(kc == 7),
                        )
                    if warm:
                        warm_mm()

            def q_fold(qps, mop, evac_dve):
                qstage = work.tile([128, 1024], F16, tag="qstage", name="qstage",
                                   bufs=1)
                if evac_dve:
                    nc.vector.tensor_copy(qstage[:, :], qps[:, :])
                else:
                    nc.scalar.activation(qstage[:, :], qps[:, :], COPY)
                # fold: qt2[64u+d, 2048p+128t'+r] = qstage[64par+d, 512half+128p+r]
                # with t' = 4*mop + 2*half + par
                qt4 = qt2[:, :].rearrange("a (p c) -> a p c", c=2048)
                for half in range(2):
                    src3 = qstage[:, 512 * half:512 * (half + 1)].rearrange(
                        "a (p r) -> a p r", r=128)
                    for par in range(2):
                        tp_ = 4 * mop + 2 * half + par
                        for u in range(2):
                            nc.vector.tensor_copy(
                                qt4[64 * u:64 * u + 64, :,
                                    128 * tp_:128 * (tp_ + 1)],
                                src3[64 * par:64 * par + 64, :, :])

            def q_chunk(mop, evac_dve=False, warm=False):
                qps = ps.tile([128, 1024], F32, tag="stt", name=f"qps{mop}")
                q_mms(qps, mop, range(8), warm)
                q_fold(qps, mop, evac_dve)

            def k_mms(kps, mop, kcs, warm):
                for kc in kcs:
                    for half in range(2):
                        nc.tensor.matmul(
                            kps[:, 512 * half:512 * (half + 1)],
                            lhsT=wk_st[mop][:, 256 * kc + 128 * half:
                                            256 * kc + 128 * (half + 1)],
                            rhs=xkst[:, 512 * kc:512 * (kc + 1)],
                            start=(kc == 0), stop=(kc == 7),
                        )
                    if warm:
                        warm_mm()

            def k_evac(kps, mop, evac_dve):
                if evac_dve:
                    nc.vector.tensor_copy(kt[:, 1024 * mop:1024 * (mop + 1)],
                                          kps[:, :])
                else:
                    nc.scalar.activation(kt[:, 1024 * mop:1024 * (mop + 1)],
                                         kps[:, :], COPY)

            def k_chunk(mop, evac_dve=False, warm=False):
                kps = ps.tile([128, 1024], F32, tag="stt", name=f"kps{mop}")
                k_mms(kps, mop, range(8), warm)
                k_evac(kps, mop, evac_dve)

            def v_mms(vps, p, kcs):
                for kc in kcs:
                    for oc in range(2):
                        nc.tensor.matmul(
                            vps[:, 512 * oc:512 * (oc + 1)],
                            lhsT=xvst[:, 512 * kc + 128 * p:
                                       512 * kc + 128 * (p + 1)],
                            rhs=wvst[:, 1024 * kc + 512 * oc:
                                     1024 * kc + 512 * (oc + 1)],
                            start=(kc == 0), stop=(kc == 7),
                        )

            def v_evac(vps, p, evac_dve):
                src3 = vps[:, :].rearrange("a (t d) -> a t d", d=64)
                dst3 = vaug[p][:, :].rearrange("a (t c) -> a t c", c=128)
                if evac_dve:
                    nc.vector.tensor_copy(dst3[:, :, 64:128], src3)
                else:
                    nc.scalar.activation(dst3[:, :, 64:128], src3, COPY)

            def v_pair(p, evac_dve=False, tag="stt"):
                vps = ps.tile([128, 1024], F32, tag=tag, name=f"vps{p}")
                v_mms(vps, p, range(8))
                v_evac(vps, p, evac_dve)

            def final_mms(psF, p, tps):
                for tp_ in tps:
                    for oc in range(2):
                        nc.tensor.matmul(
                            psF[:, 512 * oc:512 * (oc + 1)],
                            lhsT=stk[p][:, 128 * tp_:128 * (tp_ + 1)],
                            rhs=wo_sb[:, 1024 * tp_ + 512 * oc:
                                      1024 * tp_ + 512 * (oc + 1)],
                            start=(tp_ == 0), stop=(tp_ == 7),
                        )

            def final_emit(psF, p):
                # evacuate on ACT: it sits at a sweep boundary where the exp
                # stream naturally idles behind the DVE normalize
                outsb = work.tile([128, 1024], F16, tag="outsb", name="outsb",
                                  bufs=1)
                nc.scalar.activation(outsb[:, :], psF[:, :], COPY)
                nc.sync.dma_start(out=out_d[128 * p:128 * (p + 1), :],
                                  in_=outsb[:, :])

            def final_proj(p):
                psF = ps.tile([128, 1024], F32, tag="big", name=f"psF{p}")
                final_mms(psF, p, range(8))
                final_emit(psF, p)

            # ---------------- attention stream ----------------
            # Global PV queue: entries carry their psO, so a sweep's last PV
            # matmuls and its normalize drain DURING the next sweep instead
            # of serializing at the boundary (the exp stream never pauses).
            pv_q = []

            def normalize(psO, p, qh):
                # psO rows 0:63 hold the broadcast denominator, rows 64:127
                # hold O^T.  Write stack2 repacked by t-parity.  Processed
                # per sc-half so the next sweep's PV (which overwrites this
                # psO slot bank by bank) unblocks after half the chain.
                for sc in range(2):
                    recip = work.tile([64, 512], F32, tag="recip", name="recip",
                                      bufs=1)
                    cs = slice(512 * sc, 512 * (sc + 1))
                    nc.vector.reciprocal_approx_fast(recip[:, :], psO[0:64, cs])
                    src3 = psO[64:128, cs].rearrange("a (tl r) -> a tl r", r=128)
                    rec3 = recip[:, :].rearrange("a (tl r) -> a tl r", r=128)
                    for par in range(2):
                        dst3 = stk[p][64 * par:64 * par + 64,
                                      512 * qh + 256 * sc:
                                      512 * qh + 256 * (sc + 1)].rearrange(
                            "a (i r) -> a i r", r=128)
                        nc.vector.tensor_mul(dst3, src3[:, par::2, :],
                                             rec3[:, par::2, :])

            def drain_pv():
                psO, p, qh, t, pm = pv_q.pop(0)
                for sc in range(2):
                    nc.tensor.matmul(
                        psO[:, 512 * sc:512 * (sc + 1)],
                        lhsT=vaug[p][:, 128 * t:128 * (t + 1)],
                        rhs=pm[:, 512 * sc:512 * (sc + 1)],
                        start=(t == 0), stop=(t == 15),
                    )
                if t == 15:
                    normalize(psO, p, qh)

            def sweep(p, qh, inject=None):
                """One (pair, q-half) pass: 8 t-pair groups; PV/normalize of
                the previous sweep and injected phase-1 work ride between
                groups."""
                inject = inject or {}
                psO = ps.tile([128, 1024], F32, tag="big", name=f"psO{p}{qh}")
                for g in range(8):
                    t0, t1 = 2 * g, 2 * g + 1
                    stt0 = ps.tile([128, 1024], F32, tag="stt", name="stt0")
                    stt1 = ps.tile([128, 1024], F32, tag="stt", name="stt1")
                    # complete t0's matmuls before t1's: exp(t0) of the next
                    # group gates only on t0, so this keeps ACT saturated
                    # (t0's sc1 and t1's sc0 still sit on opposite row tiles)
                    for t, stt in ((t0, stt0), (t1, stt1)):
                        for sc in range(2):
                            par, mo = t % 2, t // 2
                            nc.tensor.matmul(
                                stt[:, 512 * sc:512 * (sc + 1)],
                                lhsT=kt[64 * par:64 * par + 64,
                                        512 * mo + 128 * p:512 * mo + 128 * (p + 1)],
                                rhs=qt2[64 * par:64 * par + 64,
                                        2048 * p + 1024 * qh + 512 * sc:
                                        2048 * p + 1024 * qh + 512 * (sc + 1)],
                                start=True, stop=True,
                            )
                    for t, stt in ((t0, stt0), (t1, stt1)):
                        praw = work.tile([128, 1024], F16, tag="praw",
                                         name="praw", bufs=2)
                        nc.scalar.activation(praw[:, :], stt[:, :], EXP,
                                             scale=EXP_SCALE)
                        pm = work.tile([128, 1024], F16, tag="pm", name="pm",
                                       bufs=3)
                        nc.vector.tensor_mul(
                            pm[:, :], praw[:, :],
                            mask_sb[t][:, 1024 * qh:1024 * (qh + 1)])
                        pv_q.append((psO, p, qh, t, pm))
                    while len(pv_q) > 2:
                        drain_pv()
                    for fn in inject.get(g, ()):
                        fn()

            # ---------------- emission schedule ----------------
            q_chunk(0, warm=True)
            q_chunk(1, warm=True)
            # junk matmuls gated on the K input DMAs: they pace the PE
            # through the DMA-bound stretch so the HAM clock stays warm
            for kc in range(8):
                nc.tensor.matmul(warm_ps[:, 0:512], lhsT=junk[:, 0:128],
                                 rhs=xkst[:, 512 * kc:512 * (kc + 1)],
                                 start=True, stop=True)
            k_chunk(0, warm=True)
            k_chunk(1, warm=True)
            for kc in range(0, 8, 2):
                nc.tensor.matmul(warm_ps[:, 0:512], lhsT=junk[:, 0:128],
                                 rhs=xvst[:, 512 * kc:512 * (kc + 1)],
                                 start=True, stop=True)
            v_pair(0)

            # split phase-1 remainders into half-chunks so no injection puts
            # more than ~8 matmuls between attention groups
            ph1 = {}

            def kA(mop):
                ph1[("k", mop)] = ps.tile([128, 1024], F32, tag="stt",
                                          name=f"kps{mop}")
                k_mms(ph1[("k", mop)], mop, range(4), False)

            def kB(mop):
                k_mms(ph1[("k", mop)], mop, range(4, 8), False)
                k_evac(ph1[("k", mop)], mop, True)

            def qA(mop):
                ph1[("q", mop)] = ps.tile([128, 1024], F32, tag="stt",
                                          name=f"qps{mop}")
                q_mms(ph1[("q", mop)], mop, range(4), False)

            def qB(mop):
                q_mms(ph1[("q", mop)], mop, range(4, 8), False)
                q_fold(ph1[("q", mop)], mop, True)

            def vA(p):
                ph1[("v", p)] = ps.tile([128, 1024], F32, tag="big",
                                        name=f"vps{p}")
                v_mms(ph1[("v", p)], p, range(4))

            def vB(p):
                v_mms(ph1[("v", p)], p, range(4, 8))
                v_evac(ph1[("v", p)], p, True)

            def fA(p):
                ph1[("f", p)] = ps.tile([128, 1024], F32, tag="big",
                                        name=f"psF{p}")
                final_mms(ph1[("f", p)], p, range(4))

            def fB(p):
                final_mms(ph1[("f", p)], p, range(4, 8))
                final_emit(ph1[("f", p)], p)

            sweep(0, 0, inject={
                1: [lambda: kA(2)], 2: [lambda: kB(2), lambda: qA(2)],
                3: [lambda: qB(2), lambda: kA(3)],
                4: [lambda: kB(3), lambda: qA(3)],
                5: [lambda: qB(3), lambda: vA(1)], 6: [lambda: vB(1)],
            })
            sweep(0, 1, inject={1: [lambda: vA(2)], 2: [lambda: vB(2)]})
            sweep(1, 0, inject={1: [lambda: fA(0)], 2: [lambda: fB(0)]})
            sweep(1, 1, inject={1: [lambda: vA(3)], 2: [lambda: vB(3)]})
            sweep(2, 0, inject={1: [lambda: fA(1)], 2: [lambda: fB(1)]})
            sweep(2, 1)
            sweep(3, 0, inject={1: [lambda: fA(2)], 2: [lambda: fB(2)]})
            # pair-3 final projection is split across the two q-halves: tp
            # 0..3 only touch stack columns written by the qh=0 normalize,
            # which drains early in sweep(3,1).
            sweep(3, 1, inject={2: [lambda: fA(3)]})
            while pv_q:
                drain_pv()
            fB(3)

    nc.finalize()
    return nc


def build_in_maps(inputs):
    f = np.float32
    q = np.asarray(inputs["q"], f)
    k = np.asarray(inputs["k"], f)
    v = np.asarray(inputs["v"], f)
    mask = np.asarray(inputs["mask"])
    w_q = np.asarray(inputs["w_q"], f)
    w_k = np.asarray(inputs["w_k"], f)
    w_v = np.asarray(inputs["w_v"], f)
    w_o = np.asarray(inputs["w_o"], f)

    wq = np.ascontiguousarray(w_q.T).astype(np.float16).reshape(8, 128, 1024)
    wk = np.ascontiguousarray(w_k.T).astype(np.float16).reshape(8, 128, 1024)
    wv = np.ascontiguousarray(w_v.T).astype(np.float16).reshape(8, 128, 1024)
    wo2 = np.ascontiguousarray(
        w_o.T.reshape(8, 2, 64, 1024).transpose(1, 2, 0, 3).reshape(128, 8192)
    ).astype(np.float16)
    maskc = []
    for b in range(B):
        mt_ = (~mask[b]).T.astype(np.uint8)            # [k', q'] keep-flags
        mp = mt_.reshape(S, 128, 16).transpose(0, 2, 1).reshape(S, S)
        maskc.append(np.ascontiguousarray(mp))

    in_maps = []
    for c in range(N_CORES):
        b, sb = c // 4, c % 4
        rows = slice(CORE_ROWS * sb, CORE_ROWS * (sb + 1))
        xq = np.ascontiguousarray(q[b, rows].T).astype(np.float16).reshape(8, 128, 512)
        xk = np.ascontiguousarray(k[b, rows].T).astype(np.float16).reshape(8, 128, 512)
        xv = np.ascontiguousarray(v[b, rows].T).astype(np.float16).reshape(8, 128, 512)
        in_maps.append({
            "xq": xq, "wq": wq, "xk": xk, "wk": wk, "xv": xv, "wv": wv,
            "wo2": wo2, "maskc": maskc[b],
        })
    return in_maps


def kernel(q, k, v, mask, w_q, w_k, w_v, w_o):
    global _NC
    if _NC is None:
        _NC = _build_program()

    in_maps = build_in_maps(dict(q=q, k=k, v=v, mask=mask,
                                 w_q=w_q, w_k=w_k, w_v=w_v, w_o=w_o))
    res = run_bass_kernel_spmd(_NC, in_maps, list(range(N_CORES))).results

    out = np.empty((B, S, D), dtype=np.float32)
    for c in range(N_CORES):
        b, sb = c // 4, c % 4
        out[b, CORE_ROWS * sb:CORE_ROWS * (sb + 1)] = \
            res[c]["out"].astype(np.float32)
    return out
